# revision 43
# baseline (speedup 1.0000x reference)
"""Trainium2 Bass kernel for a 2-layer GCN + linear classifier (PyG GCNConv style).

Self-contained: hardcodes the 8-core sharding strategy; all graph/index
preprocessing is host-side numpy, all FLOPs on x run on device.

Sharding: nodes are split into 8 contiguous canonical blocks (one per core,
padded to a multiple of 128). Per GCN layer each core computes its block's
dense transform (bf16 PE matmuls, f32 PSUM), pre-scales rows by dinv, and two
AllGathers (A fired mid-matmul) materialize the full bf16 node table in every
core's HBM. Each core aggregates its own destinations' in-edges (self-loops
included as real edges) with gpsimd dma_gather (256B bf16 rows, <=1024
indices per instruction — hard ucode descriptor-ring limit — over 4 SWDGE
queues). Slab reduction rotates 2:1 between DVE bf16 tree-adds and PE
identity-matmul PSUM accumulation; gather instructions stream slabs across
tile boundaries so every instruction carries a full 1024 indices. Two passes
per layer split by physical table half so gather indices fit int16 (half A
kept as small as the int16 range allows so the first AllGather — and hence
the first gather — starts sooner); destinations are degree-sorted per
(core, layer, pass) to minimize slab padding. Each layer's result is merged
in PASS-B ORDER: pass-A partials round-trip through DRAM and are re-gathered
directly into B-order (composed permutation, emitted inside the pass-B gather
stream so it is off the critical path). Layer 2 consumes B1-ordered h; its
gather indices are remapped on the host. The final output leaves the device
in B2 order and the host unpermutes during unsharding. The bottleneck is the
Pool engine's dma_gather ucode at ~2.15 ns/index (engine-serial).
"""

import sys
import types

import numpy as np


def _setup_env():
    if "/opt/trn_rl_repo" not in sys.path:
        sys.path.insert(0, "/opt/trn_rl_repo")
    if "antenv.axon_hooks" not in sys.modules:
        try:
            from trn_agent_boot.trn_boot import _ntff_profile_via_ctypes

            _hook = _ntff_profile_via_ctypes("/opt/axon/libaxon_pjrt.so")
        except Exception:
            _hook = None
        _mod = types.ModuleType("antenv.axon_hooks")
        _mod.get_axon_ntff_profile_hook = lambda: _hook
        _mod.set_axon_ntff_profile_hook = lambda h: None
        sys.modules["antenv.axon_hooks"] = _mod


_setup_env()

import ml_dtypes  # noqa: E402
from concourse import bacc, bass, mybir, tile  # noqa: E402
import concourse.bass_utils as bass_utils  # noqa: E402
from concourse.bass_utils import run_bass_kernel_spmd  # noqa: E402
from concourse.masks import make_identity  # noqa: E402

bass_utils.upload_artifacts = lambda tmpdir: tmpdir

# --- queue-aware DMASW semaphore lane assignment -----------------------------
# Tile assigns Pool-engine DMA instructions to the 8 DMASW semaphore lanes
# round-robin in *scheduled* order, but each lane gets locked to the SWDGE
# queue of the first instruction using it. With multi-queue dma_gather this
# races; pin each queue to its own lane subset instead.
import concourse.tile_sem_assignment as _tsa  # noqa: E402
from concourse.bass_isa import UserSyncedRemoteDMADescs as _URD  # noqa: E402
from concourse.tile_sem_assignment import DMAInst as _DMAInst  # noqa: E402

_orig_assign_tick = _tsa.TileClockTick._assign_tick


def _queue_aware_assign_tick(self, inst):
    if (
        isinstance(inst, _DMAInst)
        and not isinstance(inst, _URD)
        and inst.engine == mybir.EngineType.Pool
    ):
        q = getattr(inst, "queue_num", 0) or 0
        lanes = max(1, self.swdge_sem_count // NQ)
        rot = self.__dict__.setdefault("_q_lane_rot", {})
        r = rot.get(q, 0)
        self.next_sw_dma_idx = (q * lanes + r) % self.swdge_sem_count
        rot[q] = (r + 1) % lanes
    return _orig_assign_tick(self, inst)


_tsa.TileClockTick._assign_tick = _queue_aware_assign_tick
# -----------------------------------------------------------------------------

import os  # noqa: E402

N_CORES = 8
P = 128
CHUNK = 8   # gather slabs (of 128 rows) per dma_gather instruction
            # (hard ucode limit: 1024 descriptors per ring/instruction)
NQ = int(os.environ.get("KNQ", "4"))  # SWDGE queues
GBUFS = int(os.environ.get("KGBUFS", "8"))
SCAP = int(os.environ.get("KSCAP", "24"))  # max staging slabs per group
PEROT = int(os.environ.get("KPEROT", "3"))  # every PEROT-th tile reduced on PE

dt = mybir.dt
BF16 = ml_dtypes.bfloat16


# ----------------------------------------------------------------------------
# Host-side preprocessing
# ----------------------------------------------------------------------------

def _wrap16(flat: np.ndarray) -> np.ndarray:
    """Lay out an index list in dma_gather's [128, n/16] wrapped format."""
    n = flat.shape[0]
    assert n % 16 == 0
    w = flat.reshape(n // 16, 16).T.astype(np.int16)  # [16, n//16]
    return np.tile(w, (8, 1))  # replicate across the 8 groups of 16 partitions


def _build_grid(s, d_pos, Kg, off, zrow, BLK, P_=P):
    """Slab grid [sumK, P]: grid[off[t]+k, lane] = k-th source of the dst at
    sorted position t*P+lane; unfilled slots -> zrow."""
    sumK = int(Kg.sum())
    grid = np.full((sumK, P_), zrow, dtype=np.int64)
    order = np.argsort(d_pos, kind="stable")
    pos_s = d_pos[order]
    s_s = s[order]
    counts = np.bincount(pos_s, minlength=BLK)
    starts = np.concatenate([[0], np.cumsum(counts)[:-1]])
    k = np.arange(len(pos_s)) - starts[pos_s]
    tile_i = pos_s // P_
    lane = pos_s % P_
    grid[off[tile_i] + k, lane] = s_s
    return grid


def _preprocess(x, edge_index, W1, b1, W2, b2, Wfc, bfc):
    N, IN = x.shape
    HID = W1.shape[1]
    CLS = Wfc.shape[1]
    assert IN % P == 0 and HID == P

    BLK_RAW = -(-N // N_CORES)            # nodes per core before padding
    BLK = -(-BLK_RAW // P) * P            # padded block size
    assert BLK_RAW + 2 <= BLK, "need >=2 pad slots per block"
    NPAD = N_CORES * BLK
    MT = BLK // P
    # half-A as small as the int16 index range allows (both halves must stay
    # under 32768 physical rows): a small half A starts gathers sooner
    MTA = max(0, MT - (32768 // (N_CORES * P) - 1), MT // 2 - 13)
    MTA = min(MTA, MT // 2)
    HA = MTA * P                          # rows per block in half A
    HB = BLK - HA
    NROWSA = N_CORES * HA                 # physical half-A table rows
    NROWSB = N_CORES * HB
    assert NROWSA < 32768 and NROWSB < 32768

    src = edge_index[0].astype(np.int64)
    dst = edge_index[1].astype(np.int64)

    deg = np.bincount(dst, minlength=N).astype(np.float64) + 1.0
    dinv = (1.0 / np.sqrt(deg)).astype(np.float32)
    dinv_c = np.zeros(NPAD, dtype=np.float32)
    all_ids = np.arange(N, dtype=np.int64)
    # block-local slot: j=0 reserved as a pad (deg 0 -> sorts to position 0),
    # reals at j in [1, BLK_RAW], remaining pads at the tail. Slot BLK-1 is
    # force-sorted LAST so every pass order has a zero row in half B.
    canon = (all_ids // BLK_RAW) * BLK + 1 + (all_ids % BLK_RAW)
    dinv_c[canon] = dinv

    def phys(c):
        r = c // BLK
        j = c % BLK
        return np.where(j < HA, r * HA + j, NROWSA + r * HB + (j - HA))

    # canonical edge list WITH self-loops (dst-major structures built per core)
    src_c = (src // BLK_RAW) * BLK + 1 + (src % BLK_RAW)
    dst_c = (dst // BLK_RAW) * BLK + 1 + (dst % BLK_RAW)
    loop_c = canon
    src_all = np.concatenate([src_c, loop_c])
    dst_all = np.concatenate([dst_c, loop_c])

    # per-core edge lists in canonical-local dst coords
    core_edges = []
    for r in range(N_CORES):
        lo, hi = r * BLK, (r + 1) * BLK
        m = (dst_all >= lo) & (dst_all < hi)
        core_edges.append((src_all[m], dst_all[m] - lo))

    def layer_structs(src_phys_of):
        """Build per-core pass structures for one layer.

        src_phys_of: canonical src id -> physical table row for this layer.
        Returns (per_core, KA, KB) where per_core[r] = dict with grids built
        later (needs global K), plus permB/invpermA maps.
        """
        per_core = []
        KAg = np.zeros(MT, dtype=np.int64)
        KBg = np.zeros(MT, dtype=np.int64)
        for r in range(N_CORES):
            s_can, d_loc = core_edges[r]
            s_phys = src_phys_of(s_can)
            passes = []
            for half in (0, 1):
                pm = (s_phys >= NROWSA) if half else (s_phys < NROWSA)
                s_p = s_phys[pm] - half * NROWSA
                d_p = d_loc[pm]
                degp = np.bincount(d_p, minlength=BLK)
                key = degp.copy()
                key[BLK - 1] = 1 << 30      # force tail pad slot to sort last
                perm = np.argsort(key, kind="stable")  # perm[pos] = local id
                invperm = np.empty(BLK, dtype=np.int64)
                invperm[perm] = np.arange(BLK)
                Kt = degp[perm].reshape(MT, P).max(axis=1)
                passes.append(dict(s=s_p, d=d_p, perm=perm, invperm=invperm,
                                   Kt=Kt))
            per_core.append(passes)
            KAg = np.maximum(KAg, per_core[r][0]["Kt"])
            KBg = np.maximum(KBg, per_core[r][1]["Kt"])
        return per_core, KAg, KBg

    # ---- layer 1: table in canonical-phys layout ----
    KL1 = np.zeros(MT, dtype=np.int64)  # local pass disabled
    offL1 = np.concatenate([[0], np.cumsum(KL1)[:-1]])
    local_structs = [dict(s=np.zeros(0, np.int64), d=np.zeros(0, np.int64),
                          invperm=np.arange(BLK)) for _ in range(N_CORES)]
    pc1, KA1, KB1 = layer_structs(lambda c: phys(c))
    # ---- layer 2: table in B1-order layout (per owner core) ----
    # src canonical c -> owner core rc, slot j -> B1 position -> phys row
    posB1 = np.empty(NPAD, dtype=np.int64)
    for r in range(N_CORES):
        posB1[r * BLK:(r + 1) * BLK] = r * BLK + pc1[r][1]["invperm"]
    pc2, KA2, KB2 = layer_structs(lambda c: phys(posB1[c]))

    offA1 = np.concatenate([[0], np.cumsum(KA1)[:-1]])
    offB1 = np.concatenate([[0], np.cumsum(KB1)[:-1]])
    offA2 = np.concatenate([[0], np.cumsum(KA2)[:-1]])
    offB2 = np.concatenate([[0], np.cumsum(KB2)[:-1]])

    # x blocks, pre-tiled for the PE: [MT*P, KC*P] where row m*P+k holds
    # tile m's moving operand row k (feature chunk-major, node minor)
    KC = IN // P
    xt_blocks = []
    for r in range(N_CORES):
        lo = r * BLK_RAW
        hi = min(N, (r + 1) * BLK_RAW)
        xb = np.zeros((BLK, IN), dtype=np.float32)
        if hi > lo:
            xb[1:1 + hi - lo] = x[lo:hi]
        xtt = xb.reshape(MT, P, KC, P).transpose(0, 3, 2, 1).reshape(
            MT * P, KC * P)
        xt_blocks.append(np.ascontiguousarray(xtt).astype(BF16))

    b1r = np.tile(np.asarray(b1, np.float32)[None, :], (P, 1))
    b2r = np.tile(np.asarray(b2, np.float32)[None, :], (P, 1))
    bfcr = np.tile(np.asarray(bfc, np.float32)[None, :], (P, 1))
    w1 = np.asarray(W1, np.float32).astype(BF16)
    w2 = np.asarray(W2, np.float32).astype(BF16)
    wfc = np.asarray(Wfc, np.float32).astype(BF16)

    in_maps = []
    unperm = []
    for r in range(N_CORES):
        dv_blk = dinv_c[r * BLK:(r + 1) * BLK]
        permB1 = pc1[r][1]["perm"]
        permB2 = pc2[r][1]["perm"]
        # dinv variants: canonical (L1 pre-scale); B1 (L1 post + L2 pre);
        # B2 (L2 post)
        dv3 = np.stack([dv_blk, dv_blk[permB1], dv_blk[permB2]], axis=0)
        dv3 = dv3.reshape(3, MT, P).transpose(2, 0, 1)  # [P, 3, MT]

        def grids_for(pcr, K_A, K_B, oA, oB, zrowA, zrowB):
            pa, pb = pcr
            # dst coords -> sorted positions for this pass
            gA = _build_grid(pa["s"], pa["invperm"][pa["d"]], K_A, oA,
                             zrowA, BLK)
            gB = _build_grid(pb["s"], pb["invperm"][pb["d"]], K_B, oB,
                             zrowB, BLK)
            if gA.shape[0] == 0:  # keep DRAM tensors non-empty
                gA = np.full((1, P), zrowA, dtype=np.int64)
            if gB.shape[0] == 0:
                gB = np.full((1, P), zrowB, dtype=np.int64)
            return gA, gB

        # layer-1 zero rows: canonical pads (j=0 of own block in half A;
        # tail pad slot BLK-2 lands in half B)
        z1a = int(phys(np.array([r * BLK]))[0])
        z1b = int(phys(np.array([r * BLK + BLK - 2]))[0]) - NROWSA
        assert BLK - 2 >= HA or HA == 0
        gA1, gB1 = grids_for(pc1[r], KA1, KB1, offA1, offB1,
                             z1a if HA > 0 else 0, z1b)
        # layer-2 zero rows: position 0 (j=0 pad sorts first) in half A;
        # position BLK-1 (forced pad) in half B
        z2a = int(phys(np.array([r * BLK]))[0])
        z2b = int(phys(np.array([r * BLK + BLK - 1]))[0]) - NROWSA
        gA2, gB2 = grids_for(pc2[r], KA2, KB2, offA2, offB2,
                             z2a if HA > 0 else 0, z2b)

        # composed canon: pd_A (A-order) -> B-order
        mapAB1 = pc1[r][0]["invperm"][permB1]
        mapAB2 = pc2[r][0]["invperm"][permB2]
        ls = local_structs[r]
        gL1 = _build_grid(ls["s"], ls["invperm"][ls["d"]], KL1, offL1, 0, BLK)
        if gL1.shape[0] == 0:
            gL1 = np.full((1, P), 0, dtype=np.int64)
        mapLB1 = ls["invperm"][permB1]

        in_maps.append({
            "xt": xt_blocks[r],
            "w1": w1, "w2": w2, "wfc": wfc,
            "b1r": b1r, "b2r": b2r, "bfcr": bfcr,
            "dinv3": np.ascontiguousarray(dv3.astype(np.float32)),
            "idxa1": np.ascontiguousarray(_wrap16(gA1.reshape(-1))),
            "idxb1": np.ascontiguousarray(_wrap16(gB1.reshape(-1))),
            "idxa2": np.ascontiguousarray(_wrap16(gA2.reshape(-1))),
            "idxb2": np.ascontiguousarray(_wrap16(gB2.reshape(-1))),
            "mapab1": np.ascontiguousarray(_wrap16(mapAB1)),
            "mapab2": np.ascontiguousarray(_wrap16(mapAB2)),
            "idxl1": np.ascontiguousarray(_wrap16(gL1.reshape(-1))),
            "mapl1": np.ascontiguousarray(_wrap16(mapLB1)),
        })
        unperm.append(pc2[r][1]["invperm"])  # node slot -> B2 position

    meta = dict(N=N, IN=IN, HID=HID, CLS=CLS, BLK=BLK, BLK_RAW=BLK_RAW,
                NPAD=NPAD, MT=MT, MTA=MTA, NROWSA=NROWSA, NROWSB=NROWSB,
                KA1=tuple(map(int, KA1)), KB1=tuple(map(int, KB1)),
                KA2=tuple(map(int, KA2)), KB2=tuple(map(int, KB2)),
                KL1=tuple(map(int, KL1)))
    return in_maps, meta, unperm


# ----------------------------------------------------------------------------
# Device graph
# ----------------------------------------------------------------------------

class _ScalarAdder:
    """Adapter: ACT-engine copy via activation identity."""

    def __init__(self, nc):
        self.nc = nc

    def tensor_copy(self, out, a):
        self.nc.scalar.activation(out, a,
                                  mybir.ActivationFunctionType.Identity)


def _tree_reduce_range(nc, g, lo, n, out_ap, eng):
    """Sum g[:, lo:lo+n, :] slabs; final level writes into out_ap."""
    if n == 1:
        eng.tensor_copy(out_ap, g[:, lo, :])
        return
    while n > 2:
        if n % 2 == 1:
            eng.tensor_add(g[:, lo, :], g[:, lo, :], g[:, lo + n - 1, :])
            n -= 1
            if n == 2:
                break
        h = n // 2
        eng.tensor_add(g[:, lo:lo + h, :], g[:, lo:lo + h, :],
                       g[:, lo + h:lo + 2 * h, :])
        n = h
    eng.tensor_add(out_ap, g[:, lo, :], g[:, lo + 1, :])


def _build(meta):
    IN, HID, CLS = meta["IN"], meta["HID"], meta["CLS"]
    BLK, MT, MTA = meta["BLK"], meta["MT"], meta["MTA"]
    NROWSA, NROWSB = meta["NROWSA"], meta["NROWSB"]
    KS = {1: (meta["KA1"], meta["KB1"]), 2: (meta["KA2"], meta["KB2"])}
    KL = meta["KL1"]
    WA = {1: max(1, sum(meta["KA1"])), 2: max(1, sum(meta["KA2"]))}
    WB = {1: max(1, sum(meta["KB1"])), 2: max(1, sum(meta["KB2"]))}
    WL = max(1, sum(KL))
    WAmax, WBmax = max(WA.values()), max(WB.values())
    KC = IN // P

    nc = bacc.Bacc("TRN2", target_bir_lowering=False, debug=False,
                   num_devices=N_CORES, num_swdge_queues=NQ)

    xt = nc.dram_tensor("xt", [MT * P, KC * P], dt.bfloat16,
                        kind="ExternalInput")
    w1 = nc.dram_tensor("w1", [IN, HID], dt.bfloat16, kind="ExternalInput")
    w2 = nc.dram_tensor("w2", [HID, HID], dt.bfloat16, kind="ExternalInput")
    wfc = nc.dram_tensor("wfc", [HID, CLS], dt.bfloat16, kind="ExternalInput")
    b1r = nc.dram_tensor("b1r", [P, HID], dt.float32, kind="ExternalInput")
    b2r = nc.dram_tensor("b2r", [P, HID], dt.float32, kind="ExternalInput")
    bfcr = nc.dram_tensor("bfcr", [P, CLS], dt.float32, kind="ExternalInput")
    dinv3 = nc.dram_tensor("dinv3", [P, 3, MT], dt.float32,
                           kind="ExternalInput")
    idx_dram = {}
    for layer in (1, 2):
        idx_dram[(layer, 0)] = nc.dram_tensor(
            f"idxa{layer}", [P, WA[layer] * 8], dt.int16, kind="ExternalInput")
        idx_dram[(layer, 1)] = nc.dram_tensor(
            f"idxb{layer}", [P, WB[layer] * 8], dt.int16, kind="ExternalInput")
    mapab = {1: nc.dram_tensor("mapab1", [P, BLK // 16], dt.int16,
                               kind="ExternalInput"),
             2: nc.dram_tensor("mapab2", [P, BLK // 16], dt.int16,
                               kind="ExternalInput")}
    idxl1 = nc.dram_tensor("idxl1", [P, WL * 8], dt.int16,
                           kind="ExternalInput")
    mapl1 = nc.dram_tensor("mapl1", [P, BLK // 16], dt.int16,
                           kind="ExternalInput")
    out = nc.dram_tensor("out", [BLK, CLS], dt.float32, kind="ExternalOutput")

    with tile.TileContext(nc) as tc:
        with (
            tc.tile_pool(name="const", bufs=1) as cpool,
            tc.tile_pool(name="idx", bufs=1) as ipool,
            tc.tile_pool(name="big", bufs=4) as bigpool,
            tc.tile_pool(name="xload", bufs=6) as xpool,
            tc.tile_pool(name="gbuf", bufs=GBUFS) as gpool,
            tc.tile_pool(name="lhsT", bufs=3) as tpool,
            tc.tile_pool(name="ps", bufs=4, space="PSUM") as pspool,
            tc.tile_pool(name="pst", bufs=1, space="PSUM") as pstpool,
            tc.tile_pool(name="psacc", bufs=2, space="PSUM") as psaccpool,
            tc.tile_pool(name="dram", bufs=1, space="DRAM") as dpool,
        ):
            # ---- constants ----
            w1sb = cpool.tile([P, KC, HID], dt.bfloat16, tag="w1")
            nc.sync.dma_start(out=w1sb[:], in_=w1[:].rearrange("(c k) h -> k c h", k=P))
            w2sb = cpool.tile([P, HID], dt.bfloat16, tag="w2")
            nc.sync.dma_start(out=w2sb[:], in_=w2[:])
            wfcsb = cpool.tile([P, CLS], dt.bfloat16, tag="wfc")
            nc.sync.dma_start(out=wfcsb[:], in_=wfc[:])
            b1sb = cpool.tile([P, HID], dt.float32, tag="b1")
            nc.sync.dma_start(out=b1sb[:], in_=b1r[:])
            b2sb = cpool.tile([P, HID], dt.float32, tag="b2")
            nc.sync.dma_start(out=b2sb[:], in_=b2r[:])
            bfcsb = cpool.tile([P, CLS], dt.float32, tag="bfc")
            nc.sync.dma_start(out=bfcsb[:], in_=bfcr[:])
            dvsb = cpool.tile([P, 3, MT], dt.float32, tag="dinv3")
            nc.sync.dma_start(out=dvsb[:], in_=dinv3[:])
            ident = cpool.tile([P, P], dt.float32, tag="ident")
            make_identity(nc, ident[:])
            identb = cpool.tile([P, P], dt.bfloat16, tag="identb")
            nc.any.tensor_copy(identb[:], ident[:])
            mapsb = {}
            for layer in (1, 2):
                mapsb[layer] = cpool.tile([P, BLK // 16], dt.int16,
                                          tag=f"map{layer}", name="mp")
                nc.sync.dma_start(out=mapsb[layer][:], in_=mapab[layer][:])

            idxasb = ipool.tile([P, WAmax * 8], dt.int16, tag="idxa")
            idxbsb = ipool.tile([P, WBmax * 8], dt.int16, tag="idxb")
            idxlsb = ipool.tile([P, WL * 8], dt.int16, tag="idxl")
            nc.sync.dma_start(out=idxasb[:, :WA[1] * 8], in_=idx_dram[(1, 0)][:])
            nc.sync.dma_start(out=idxbsb[:, :WB[1] * 8], in_=idx_dram[(1, 1)][:])
            nc.sync.dma_start(out=idxlsb[:], in_=idxl1[:])
            maplsb = cpool.tile([P, BLK // 16], dt.int16, tag="mapl",
                                name="mpl")
            nc.sync.dma_start(out=maplsb[:], in_=mapl1[:])

            self_q = [0]
            tile_n = [0]
            xmcache = {}
            QCH = max(1, (MT + 3) // 4)
            LT = min(2 * QCH, MT)

            def do_mm(layer, m, hsA, hsB, h_prev):
                dv_pre = dvsb[:, 0 if layer == 1 else 1, :]
                tgt = (hsA[:, m, :] if m < MTA or hsB is None
                       else hsB[:, m - MTA, :])
                ps = pspool.tile([P, HID], dt.float32, tag="mm", name="ps")
                if layer == 1:
                    xm = xpool.tile([P, KC, P], dt.bfloat16, tag="x",
                                    name="xm")
                    # ACT-engine HWDGE: keeps x-loads off the sync engine's
                    # serial DMA queue (agin/table/pd writes) at startup
                    nc.scalar.dma_start(
                        out=xm[:],
                        in_=xt[m * P:(m + 1) * P, :].rearrange(
                            "p (c j) -> p c j", c=KC),
                    )
                    for c in range(KC):
                        nc.tensor.matmul(
                            ps[:], xm[:, c, :], w1sb[:, c, :],
                            start=(c == 0), stop=(c == KC - 1),
                        )
                else:
                    pst = pstpool.tile([P, P], dt.bfloat16, tag="tr",
                                       name="pst")
                    nc.tensor.transpose(pst[:], h_prev[:, m, :], identb[:])
                    hT = tpool.tile([P, P], dt.bfloat16, tag="hT", name="hT")
                    nc.any.tensor_copy(hT[:], pst[:])
                    nc.tensor.matmul(ps[:], hT[:], w2sb[:], start=True,
                                     stop=True)
                nc.vector.tensor_scalar_mul(tgt, ps[:],
                                            dv_pre[:, m:m + 1])

            def emit_ag(layer, t0, t1, nrows, hseg, hseg_tile):
                agin = dpool.tile([(t1 - t0) * P, HID], dt.bfloat16,
                                  tag=f"agin{layer}{hseg}", name="agin")
                nc.sync.dma_start(
                    out=agin[:].rearrange("(t p) h -> p t h", p=P),
                    in_=hseg_tile[:, :t1 - t0, :],
                )
                tbl = dpool.tile([nrows, HID], dt.bfloat16,
                                 tag=f"table{layer}{hseg}", name="tbl",
                                 addr_space="Shared")
                nc.gpsimd.collective_compute(
                    "AllGather",
                    mybir.AluOpType.bypass,
                    replica_groups=[list(range(N_CORES))],
                    ins=[agin[:].opt()],
                    outs=[tbl[:].opt()],
                )
                return tbl

            def canon_map(msb, pdA, out_acc):
                for c0 in range(0, MT, CHUNK):
                    cc = min(CHUNK, MT - c0)
                    nc.gpsimd.dma_gather(
                        out_ap=out_acc[:, c0:c0 + cc, :], in_ap=pdA[:],
                        idxs_ap=msb[:, c0 * 8:(c0 + cc) * 8],
                        num_idxs=cc * P, num_idxs_reg=cc * P,
                        elem_size=HID, queue_num=self_q[0] % NQ,
                    )
                    self_q[0] += 1

            def canon_for(layer, pdA, out_acc):
                canon_map(mapsb[layer], pdA, out_acc)

            def merge_tiles(t0, t1, acc, partB, hnew, dv_post, bsb,
                            acc2=None):
                c0 = t0
                while c0 < t1:
                    c1 = min(t1, c0 + QCH)
                    sl = slice(c0, c1)
                    w = c1 - c0
                    src_h = partB
                    for a in (acc, acc2):
                        if a is not None:
                            nc.vector.tensor_add(hnew[:, sl, :], a[:, sl, :],
                                                 src_h[:, sl, :])
                            src_h = hnew
                    dv3b = dv_post[:, sl].to_broadcast([P, w, HID])
                    nc.vector.tensor_tensor(hnew[:, sl, :], src_h[:, sl, :],
                                            dv3b, op=mybir.AluOpType.mult)
                    b3 = bsb[:].rearrange(
                        "p (o h) -> p o h", o=1).to_broadcast([P, w, HID])
                    nc.vector.tensor_tensor(hnew[:, sl, :], hnew[:, sl, :],
                                            b3, op=mybir.AluOpType.add)
                    nc.scalar.activation(hnew[:, sl, :], hnew[:, sl, :],
                                         mybir.ActivationFunctionType.Relu)
                    c0 = c1

            # flat slab stream: every dma_gather carries a full 1024 idx;
            # hooks inject downstream work into the emission stream
            def gather_pass(Ks, isb, tview, part, hooks=()):
                offs = np.cumsum([0] + list(Ks))
                W = int(offs[-1])
                for t in range(MT):
                    if Ks[t] == 0:
                        nc.vector.memset(part[:, t, :], 0.0)
                pend = [[int(offs[min(tl, MT)]), fn] for tl, fn in hooks]
                state = {}
                g0 = 0
                while g0 < W:
                    while pend and g0 >= pend[0][0]:
                        pend.pop(0)[1]()
                    Kg = min(SCAP, W - g0)
                    gt = gpool.tile([P, Kg, HID], dt.bfloat16, tag="g")
                    g = gt[:]
                    for s0 in range(0, Kg, CHUNK):
                        kc = min(CHUNK, Kg - s0)
                        o0 = g0 + s0
                        nc.gpsimd.dma_gather(
                            out_ap=g[:, s0:s0 + kc, :],
                            in_ap=tview,
                            idxs_ap=isb[:, o0 * 8:(o0 + kc) * 8],
                            num_idxs=kc * P,
                            num_idxs_reg=kc * P,
                            elem_size=HID,
                            queue_num=self_q[0] % NQ,
                        )
                        self_q[0] += 1
                    t_lo = int(np.searchsorted(offs, g0, "right")) - 1
                    t_hi = int(np.searchsorted(offs, g0 + Kg, "left"))
                    for t in range(t_lo, t_hi):
                        lo = max(g0, int(offs[t])) - g0
                        hi = min(g0 + Kg, int(offs[t + 1])) - g0
                        if hi <= lo:
                            continue
                        st = state.get(t)
                        if st is None:
                            use_pe = tile_n[0] % PEROT == PEROT - 1
                            tile_n[0] += 1
                            psa = (psaccpool.tile([P, HID], dt.float32,
                                                  tag="acc", name="psa")
                                   if use_pe else None)
                            st = dict(first=True, pe=use_pe, psa=psa)
                            state[t] = st
                        tile_done = g0 + hi >= int(offs[t + 1])
                        if st["pe"]:
                            for s in range(lo, hi):
                                nc.tensor.matmul(
                                    st["psa"][:], identb[:], g[:, s, :],
                                    start=(st["first"] and s == lo),
                                    stop=(tile_done and s == hi - 1),
                                )
                            if tile_done:
                                _ScalarAdder(nc).tensor_copy(
                                    part[:, t, :], st["psa"][:])
                        elif st["first"]:
                            _tree_reduce_range(nc, g, lo, hi - lo,
                                               part[:, t, :], nc.vector)
                        else:
                            tmp = tpool.tile([P, P], dt.bfloat16, tag="gtmp")
                            _tree_reduce_range(nc, g, lo, hi - lo,
                                               tmp[:, :HID], nc.vector)
                            nc.vector.tensor_add(part[:, t, :],
                                                 part[:, t, :], tmp[:, :HID])
                        st["first"] = False
                    g0 += Kg
                for _, fn in pend:
                    fn()

            # ===== layer 1: dense + AGs (split hs so AG-A only depends
            # on half-A matmuls) =====
            tables1 = []
            if MTA > 0:
                hs1A = bigpool.tile([P, MTA, HID], dt.bfloat16, tag="big",
                                    name="hs1A")
            else:
                hs1A = None
            hs1B = bigpool.tile([P, MT - MTA, HID], dt.bfloat16, tag="big",
                                name="hs1B")
            if MTA > 0:
                for m in range(MTA):
                    do_mm(1, m, hs1A, hs1B, None)
                tables1.append(emit_ag(1, 0, MTA, NROWSA, 0, hs1A))
            else:
                tables1.append(None)
            for m in range(MTA, MT):
                do_mm(1, m, hs1A if hs1A is not None else hs1B, hs1B, None)
            tables1.append(emit_ag(1, MTA, MT, NROWSB, 1, hs1B))

            # ===== layer 1: gather passes =====
            KA, KB = KS[1]
            have_A1 = NROWSA > 0 and sum(KA) > 0
            if have_A1:
                partA1 = bigpool.tile([P, MT, HID], dt.bfloat16, tag="big",
                                      name="partA1")
                gather_pass(KA, idxasb, tables1[0][:], partA1)
                pdA1 = dpool.tile([BLK, HID], dt.bfloat16, tag="pdA1",
                                  name="pdA1")
                nc.sync.dma_start(
                    out=pdA1[:].rearrange("(t p) h -> p t h", p=P),
                    in_=partA1[:])
            partB1 = bigpool.tile([P, MT, HID], dt.bfloat16, tag="big",
                                  name="partB1")
            hnew1 = bigpool.tile([P, MT, HID], dt.bfloat16, tag="big",
                                 name="hnew1")
            accA2_1 = [None]
            hs2h = []
            tables2 = []

            def mid1():
                if have_A1:
                    accA2_1[0] = bigpool.tile([P, MT, HID], dt.bfloat16,
                                              tag="big", name="accA2x")
                    canon_for(1, pdA1, accA2_1[0])

            def late1():
                merge_tiles(0, LT, accA2_1[0], partB1, hnew1,
                            dvsb[:, 1, :], b1sb)
                if MTA > 0 and LT >= MTA:
                    hs2A = bigpool.tile([P, MTA, HID], dt.bfloat16,
                                        tag="big", name="hs2A")
                    hs2h.append(hs2A)
                    for m in range(MTA):
                        do_mm(2, m, hs2A, None, hnew1)
                    tables2.append(emit_ag(2, 0, MTA, NROWSA, 0, hs2A))

            gather_pass(KB, idxbsb, tables1[1][:], partB1,
                        hooks=((8, mid1), (LT, late1)))

            # prefetch layer-2 indices (in-place overwrite)
            nc.sync.dma_start(out=idxasb[:, :WA[2] * 8],
                              in_=idx_dram[(2, 0)][:])
            nc.sync.dma_start(out=idxbsb[:, :WB[2] * 8],
                              in_=idx_dram[(2, 1)][:])

            merge_tiles(LT, MT, accA2_1[0], partB1, hnew1, dvsb[:, 1, :],
                        b1sb)
            if not hs2h:
                if MTA > 0:
                    hs2A = bigpool.tile([P, MTA, HID], dt.bfloat16,
                                        tag="big", name="hs2Ab")
                    hs2h.append(hs2A)
                    for m in range(MTA):
                        do_mm(2, m, hs2A, None, hnew1)
                    tables2.append(emit_ag(2, 0, MTA, NROWSA, 0, hs2A))
                else:
                    tables2.append(None)
            hs2B = bigpool.tile([P, MT - MTA, HID], dt.bfloat16, tag="big",
                                name="hs2B")
            for m in range(MTA, MT):
                do_mm(2, m, hs2h[0] if hs2h else hs2B, hs2B, hnew1)
            tables2.append(emit_ag(2, MTA, MT, NROWSB, 1, hs2B))

            # ===== layer 2: gather passes + fc =====
            KA, KB = KS[2]
            have_A2 = NROWSA > 0 and sum(KA) > 0
            if have_A2:
                partA2 = bigpool.tile([P, MT, HID], dt.bfloat16, tag="big",
                                      name="partA2")
                gather_pass(KA, idxasb, tables2[0][:], partA2)
                pdA2 = dpool.tile([BLK, HID], dt.bfloat16, tag="pdA2",
                                  name="pdA2")
                nc.sync.dma_start(
                    out=pdA2[:].rearrange("(t p) h -> p t h", p=P),
                    in_=partA2[:])
            partB2 = bigpool.tile([P, MT, HID], dt.bfloat16, tag="big",
                                  name="partB2")
            hnew2 = bigpool.tile([P, MT, HID], dt.bfloat16, tag="big",
                                 name="hnew2")
            outsb = bigpool.tile([P, MT, CLS], dt.float32, tag="big",
                                 name="outsb")
            accA2_2 = [None]

            def mid2():
                if have_A2:
                    accA2_2[0] = bigpool.tile([P, MT, HID], dt.bfloat16,
                                              tag="big", name="accA2y")
                    canon_for(2, pdA2, accA2_2[0])

            def fc_tiles(t0, t1):
                for m in range(t0, t1):
                    pst = pstpool.tile([P, P], dt.bfloat16, tag="tr",
                                       name="pst")
                    nc.tensor.transpose(pst[:], hnew2[:, m, :], identb[:])
                    hT = tpool.tile([P, P], dt.bfloat16, tag="hT", name="hT")
                    nc.any.tensor_copy(hT[:], pst[:])
                    ps2 = pspool.tile([P, CLS], dt.float32, tag="mm",
                                      name="ps2")
                    nc.tensor.matmul(ps2[:], hT[:], wfcsb[:], start=True,
                                     stop=True)
                    nc.vector.tensor_add(outsb[:, m, :], ps2[:], bfcsb[:])
                if t1 > t0:
                    nc.sync.dma_start(
                        out=out[:].rearrange("(t p) c -> p t c", p=P)[
                            :, t0:t1, :],
                        in_=outsb[:, t0:t1, :],
                    )

            def late2():
                merge_tiles(0, LT, accA2_2[0], partB2, hnew2,
                            dvsb[:, 2, :], b2sb)
                fc_tiles(0, LT)

            LT2 = min(max(LT, (LT + MT) // 2 + 3), MT)

            def late2b():
                merge_tiles(LT, LT2, accA2_2[0], partB2, hnew2,
                            dvsb[:, 2, :], b2sb)
                fc_tiles(LT, LT2)

            gather_pass(KB, idxbsb, tables2[1][:], partB2,
                        hooks=((8, mid2), (LT, late2), (LT2, late2b)))
            merge_tiles(LT2, MT, accA2_2[0], partB2, hnew2, dvsb[:, 2, :],
                        b2sb)
            fc_tiles(LT2, MT)

    nc.compile()
    return nc


# ----------------------------------------------------------------------------
# Entry point
# ----------------------------------------------------------------------------

_CACHE = {}


def _get_graph(meta):
    key = (meta["IN"], meta["HID"], meta["CLS"], meta["BLK"], meta["NPAD"],
           meta["KA1"], meta["KB1"], meta["KA2"], meta["KB2"], meta["KL1"])
    if key not in _CACHE:
        _CACHE[key] = _build(meta)
    return _CACHE[key]


def kernel(x, edge_index, W1, b1, W2, b2, Wfc, bfc, _want_profile=False):
    x = np.asarray(x, dtype=np.float32)
    in_maps, meta, unperm = _preprocess(
        np.asarray(x), np.asarray(edge_index), np.asarray(W1), np.asarray(b1),
        np.asarray(W2), np.asarray(b2), np.asarray(Wfc), np.asarray(bfc))
    nc = _get_graph(meta)
    res = run_bass_kernel_spmd(nc, in_maps, core_ids=list(range(N_CORES)),
                               trace=_want_profile)
    N, CLS = meta["N"], meta["CLS"]
    BLK_RAW = meta["BLK_RAW"]
    full = np.empty((N, CLS), dtype=np.float32)
    for r in range(N_CORES):
        lo = r * BLK_RAW
        hi = min(N, (r + 1) * BLK_RAW)
        if hi > lo:
            rows = unperm[r][1:1 + hi - lo]  # canonical slot j -> B2 position
            full[lo:hi] = res.results[r]["out"][rows]
    if _want_profile:
        return full, res
    return full


# revision 45
# speedup vs baseline: 1.0172x; 1.0172x over previous
"""Trainium2 Bass kernel for a 2-layer GCN + linear classifier (PyG GCNConv style).

Self-contained: hardcodes the 8-core sharding strategy; all graph/index
preprocessing is host-side numpy, all FLOPs on x run on device.

Sharding: nodes are split into 8 contiguous canonical blocks (one per core,
padded to a multiple of 128). Per GCN layer each core computes its block's
dense transform (bf16 PE matmuls, f32 PSUM), pre-scales rows by dinv, and two
AllGathers (A fired mid-matmul) materialize the full bf16 node table in every
core's HBM. Each core aggregates its own destinations' in-edges (self-loops
included as real edges) with gpsimd dma_gather (256B bf16 rows, <=1024
indices per instruction — hard ucode descriptor-ring limit — over 4 SWDGE
queues). Slab reduction rotates 2:1 between DVE bf16 tree-adds and PE
identity-matmul PSUM accumulation; gather instructions stream slabs across
tile boundaries so every instruction carries a full 1024 indices. Two passes
per layer split by physical table half so gather indices fit int16 (half A
kept as small as the int16 range allows so the first AllGather — and hence
the first gather — starts sooner); destinations are degree-sorted per
(core, layer, pass) to minimize slab padding. Each layer's result is merged
in PASS-B ORDER: pass-A partials round-trip through DRAM and are re-gathered
directly into B-order (composed permutation, emitted inside the pass-B gather
stream so it is off the critical path). Layer 2 consumes B1-ordered h; its
gather indices are remapped on the host. The final output leaves the device
in B2 order and the host unpermutes during unsharding. The bottleneck is the
Pool engine's dma_gather ucode at ~2.15 ns/index (engine-serial).
"""

import sys
import types

import numpy as np


def _setup_env():
    if "/opt/trn_rl_repo" not in sys.path:
        sys.path.insert(0, "/opt/trn_rl_repo")
    if "antenv.axon_hooks" not in sys.modules:
        try:
            from trn_agent_boot.trn_boot import _ntff_profile_via_ctypes

            _hook = _ntff_profile_via_ctypes("/opt/axon/libaxon_pjrt.so")
        except Exception:
            _hook = None
        _mod = types.ModuleType("antenv.axon_hooks")
        _mod.get_axon_ntff_profile_hook = lambda: _hook
        _mod.set_axon_ntff_profile_hook = lambda h: None
        sys.modules["antenv.axon_hooks"] = _mod


_setup_env()

import ml_dtypes  # noqa: E402
from concourse import bacc, bass, mybir, tile  # noqa: E402
import concourse.bass_utils as bass_utils  # noqa: E402
from concourse.bass_utils import run_bass_kernel_spmd  # noqa: E402
from concourse.masks import make_identity  # noqa: E402

bass_utils.upload_artifacts = lambda tmpdir: tmpdir

# --- queue-aware DMASW semaphore lane assignment -----------------------------
# Tile assigns Pool-engine DMA instructions to the 8 DMASW semaphore lanes
# round-robin in *scheduled* order, but each lane gets locked to the SWDGE
# queue of the first instruction using it. With multi-queue dma_gather this
# races; pin each queue to its own lane subset instead.
import concourse.tile_sem_assignment as _tsa  # noqa: E402
from concourse.bass_isa import UserSyncedRemoteDMADescs as _URD  # noqa: E402
from concourse.tile_sem_assignment import DMAInst as _DMAInst  # noqa: E402

_orig_assign_tick = _tsa.TileClockTick._assign_tick


def _queue_aware_assign_tick(self, inst):
    if (
        isinstance(inst, _DMAInst)
        and not isinstance(inst, _URD)
        and inst.engine == mybir.EngineType.Pool
    ):
        q = getattr(inst, "queue_num", 0) or 0
        lanes = max(1, self.swdge_sem_count // NQ)
        rot = self.__dict__.setdefault("_q_lane_rot", {})
        r = rot.get(q, 0)
        self.next_sw_dma_idx = (q * lanes + r) % self.swdge_sem_count
        rot[q] = (r + 1) % lanes
    return _orig_assign_tick(self, inst)


_tsa.TileClockTick._assign_tick = _queue_aware_assign_tick
# -----------------------------------------------------------------------------

import os  # noqa: E402

N_CORES = 8
P = 128
CHUNK = 8   # gather slabs (of 128 rows) per dma_gather instruction
            # (hard ucode limit: 1024 descriptors per ring/instruction)
NQ = int(os.environ.get("KNQ", "4"))  # SWDGE queues
GBUFS = int(os.environ.get("KGBUFS", "8"))
SCAP = int(os.environ.get("KSCAP", "24"))  # max staging slabs per group
PEROT = int(os.environ.get("KPEROT", "3"))  # every PEROT-th tile reduced on PE

dt = mybir.dt
BF16 = ml_dtypes.bfloat16


# ----------------------------------------------------------------------------
# Host-side preprocessing
# ----------------------------------------------------------------------------

def _wrap16(flat: np.ndarray) -> np.ndarray:
    """Lay out an index list in dma_gather's [128, n/16] wrapped format."""
    n = flat.shape[0]
    assert n % 16 == 0
    w = flat.reshape(n // 16, 16).T.astype(np.int16)  # [16, n//16]
    return np.tile(w, (8, 1))  # replicate across the 8 groups of 16 partitions


def _build_grid(s, d_pos, Kg, off, zrow, BLK, P_=P):
    """Slab grid [sumK, P]: grid[off[t]+k, lane] = k-th source of the dst at
    sorted position t*P+lane; unfilled slots -> zrow."""
    sumK = int(Kg.sum())
    grid = np.full((sumK, P_), zrow, dtype=np.int64)
    order = np.argsort(d_pos, kind="stable")
    pos_s = d_pos[order]
    s_s = s[order]
    counts = np.bincount(pos_s, minlength=BLK)
    starts = np.concatenate([[0], np.cumsum(counts)[:-1]])
    k = np.arange(len(pos_s)) - starts[pos_s]
    tile_i = pos_s // P_
    lane = pos_s % P_
    grid[off[tile_i] + k, lane] = s_s
    return grid


def _preprocess(x, edge_index, W1, b1, W2, b2, Wfc, bfc):
    N, IN = x.shape
    HID = W1.shape[1]
    CLS = Wfc.shape[1]
    assert IN % P == 0 and HID == P

    BLK_RAW = -(-N // N_CORES)            # nodes per core before padding
    BLK = -(-BLK_RAW // P) * P            # padded block size
    assert BLK_RAW + 2 <= BLK, "need >=2 pad slots per block"
    NPAD = N_CORES * BLK
    MT = BLK // P
    # half-A as small as the int16 index range allows (both halves must stay
    # under 32768 physical rows): a small half A starts gathers sooner
    MTA = max(0, MT - (32768 // (N_CORES * P) - 1), MT // 2 - 13)
    MTA = min(MTA, MT // 2)
    HA = MTA * P                          # rows per block in half A
    HB = BLK - HA
    NROWSA = N_CORES * HA                 # physical half-A table rows
    NROWSB = N_CORES * HB
    assert NROWSA < 32768 and NROWSB < 32768

    src = edge_index[0].astype(np.int64)
    dst = edge_index[1].astype(np.int64)

    deg = np.bincount(dst, minlength=N).astype(np.float64) + 1.0
    dinv = (1.0 / np.sqrt(deg)).astype(np.float32)
    dinv_c = np.zeros(NPAD, dtype=np.float32)
    all_ids = np.arange(N, dtype=np.int64)
    # block-local slot: j=0 reserved as a pad (deg 0 -> sorts to position 0),
    # reals at j in [1, BLK_RAW], remaining pads at the tail. Slot BLK-1 is
    # force-sorted LAST so every pass order has a zero row in half B.
    canon = (all_ids // BLK_RAW) * BLK + 1 + (all_ids % BLK_RAW)
    dinv_c[canon] = dinv

    def phys(c):
        r = c // BLK
        j = c % BLK
        return np.where(j < HA, r * HA + j, NROWSA + r * HB + (j - HA))

    # canonical edge list WITH self-loops (dst-major structures built per core)
    src_c = (src // BLK_RAW) * BLK + 1 + (src % BLK_RAW)
    dst_c = (dst // BLK_RAW) * BLK + 1 + (dst % BLK_RAW)
    loop_c = canon
    src_all = np.concatenate([src_c, loop_c])
    dst_all = np.concatenate([dst_c, loop_c])

    # per-core edge lists in canonical-local dst coords
    core_edges = []
    for r in range(N_CORES):
        lo, hi = r * BLK, (r + 1) * BLK
        m = (dst_all >= lo) & (dst_all < hi)
        core_edges.append((src_all[m], dst_all[m] - lo))

    def layer_structs(src_phys_of):
        """Build per-core pass structures for one layer.

        src_phys_of: canonical src id -> physical table row for this layer.
        Returns (per_core, KA, KB) where per_core[r] = dict with grids built
        later (needs global K), plus permB/invpermA maps.
        """
        per_core = []
        KAg = np.zeros(MT, dtype=np.int64)
        KBg = np.zeros(MT, dtype=np.int64)
        for r in range(N_CORES):
            s_can, d_loc = core_edges[r]
            s_phys = src_phys_of(s_can)
            passes = []
            for half in (0, 1):
                pm = (s_phys >= NROWSA) if half else (s_phys < NROWSA)
                s_p = s_phys[pm] - half * NROWSA
                d_p = d_loc[pm]
                degp = np.bincount(d_p, minlength=BLK)
                key = degp.copy()
                key[BLK - 1] = 1 << 30      # force tail pad slot to sort last
                perm = np.argsort(key, kind="stable")  # perm[pos] = local id
                invperm = np.empty(BLK, dtype=np.int64)
                invperm[perm] = np.arange(BLK)
                Kt = degp[perm].reshape(MT, P).max(axis=1)
                passes.append(dict(s=s_p, d=d_p, perm=perm, invperm=invperm,
                                   Kt=Kt))
            per_core.append(passes)
            KAg = np.maximum(KAg, per_core[r][0]["Kt"])
            KBg = np.maximum(KBg, per_core[r][1]["Kt"])
        return per_core, KAg, KBg

    # ---- layer 1: table in canonical-phys layout ----
    KL1 = np.zeros(MT, dtype=np.int64)  # local pass disabled
    offL1 = np.concatenate([[0], np.cumsum(KL1)[:-1]])
    local_structs = [dict(s=np.zeros(0, np.int64), d=np.zeros(0, np.int64),
                          invperm=np.arange(BLK)) for _ in range(N_CORES)]
    pc1, KA1, KB1 = layer_structs(lambda c: phys(c))
    # ---- layer 2: table in B1-order layout (per owner core) ----
    # src canonical c -> owner core rc, slot j -> B1 position -> phys row
    posB1 = np.empty(NPAD, dtype=np.int64)
    for r in range(N_CORES):
        posB1[r * BLK:(r + 1) * BLK] = r * BLK + pc1[r][1]["invperm"]
    pc2, KA2, KB2 = layer_structs(lambda c: phys(posB1[c]))

    offA1 = np.concatenate([[0], np.cumsum(KA1)[:-1]])
    offB1 = np.concatenate([[0], np.cumsum(KB1)[:-1]])
    offA2 = np.concatenate([[0], np.cumsum(KA2)[:-1]])
    offB2 = np.concatenate([[0], np.cumsum(KB2)[:-1]])

    # x blocks, pre-tiled for the PE: [MT*P, KC*P] where row m*P+k holds
    # tile m's moving operand row k (feature chunk-major, node minor)
    KC = IN // P
    xt_blocks = []
    for r in range(N_CORES):
        lo = r * BLK_RAW
        hi = min(N, (r + 1) * BLK_RAW)
        xb = np.zeros((BLK, IN), dtype=np.float32)
        if hi > lo:
            xb[1:1 + hi - lo] = x[lo:hi]
        xtt = xb.reshape(MT, P, KC, P).transpose(0, 3, 2, 1).reshape(
            MT * P, KC * P)
        xt_blocks.append(np.ascontiguousarray(xtt).astype(BF16))

    b1r = np.tile(np.asarray(b1, np.float32)[None, :], (P, 1))
    b2r = np.tile(np.asarray(b2, np.float32)[None, :], (P, 1))
    bfcr = np.tile(np.asarray(bfc, np.float32)[None, :], (P, 1))
    w1 = np.asarray(W1, np.float32).astype(BF16)
    w2 = np.asarray(W2, np.float32).astype(BF16)
    wfc = np.asarray(Wfc, np.float32).astype(BF16)

    in_maps = []
    unperm = []
    for r in range(N_CORES):
        dv_blk = dinv_c[r * BLK:(r + 1) * BLK]
        permB1 = pc1[r][1]["perm"]
        permB2 = pc2[r][1]["perm"]
        # dinv variants: canonical (L1 pre-scale); B1 (L1 post + L2 pre);
        # B2 (L2 post)
        dv3 = np.stack([dv_blk, dv_blk[permB1], dv_blk[permB2]], axis=0)
        dv3 = dv3.reshape(3, MT, P).transpose(2, 0, 1)  # [P, 3, MT]

        def grids_for(pcr, K_A, K_B, oA, oB, zrowA, zrowB):
            pa, pb = pcr
            # dst coords -> sorted positions for this pass
            gA = _build_grid(pa["s"], pa["invperm"][pa["d"]], K_A, oA,
                             zrowA, BLK)
            gB = _build_grid(pb["s"], pb["invperm"][pb["d"]], K_B, oB,
                             zrowB, BLK)
            if gA.shape[0] == 0:  # keep DRAM tensors non-empty
                gA = np.full((1, P), zrowA, dtype=np.int64)
            if gB.shape[0] == 0:
                gB = np.full((1, P), zrowB, dtype=np.int64)
            return gA, gB

        # layer-1 zero rows: canonical pads (j=0 of own block in half A;
        # tail pad slot BLK-2 lands in half B)
        z1a = int(phys(np.array([r * BLK]))[0])
        z1b = int(phys(np.array([r * BLK + BLK - 2]))[0]) - NROWSA
        assert BLK - 2 >= HA or HA == 0
        gA1, gB1 = grids_for(pc1[r], KA1, KB1, offA1, offB1,
                             z1a if HA > 0 else 0, z1b)
        # layer-2 zero rows: position 0 (j=0 pad sorts first) in half A;
        # position BLK-1 (forced pad) in half B
        z2a = int(phys(np.array([r * BLK]))[0])
        z2b = int(phys(np.array([r * BLK + BLK - 1]))[0]) - NROWSA
        gA2, gB2 = grids_for(pc2[r], KA2, KB2, offA2, offB2,
                             z2a if HA > 0 else 0, z2b)

        # composed canon: pd_A (A-order) -> B-order
        mapAB1 = pc1[r][0]["invperm"][permB1]
        mapAB2 = pc2[r][0]["invperm"][permB2]
        ls = local_structs[r]
        gL1 = _build_grid(ls["s"], ls["invperm"][ls["d"]], KL1, offL1, 0, BLK)
        if gL1.shape[0] == 0:
            gL1 = np.full((1, P), 0, dtype=np.int64)
        mapLB1 = ls["invperm"][permB1]

        in_maps.append({
            "xt": xt_blocks[r],
            "w1": w1, "w2": w2, "wfc": wfc,
            "b1r": b1r, "b2r": b2r, "bfcr": bfcr,
            "dinv3": np.ascontiguousarray(dv3.astype(np.float32)),
            "idxa1": np.ascontiguousarray(_wrap16(gA1.reshape(-1))),
            "idxb1": np.ascontiguousarray(_wrap16(gB1.reshape(-1))),
            "idxa2": np.ascontiguousarray(_wrap16(gA2.reshape(-1))),
            "idxb2": np.ascontiguousarray(_wrap16(gB2.reshape(-1))),
            "mapab1": np.ascontiguousarray(_wrap16(mapAB1)),
            "mapab2": np.ascontiguousarray(_wrap16(mapAB2)),
            "idxl1": np.ascontiguousarray(_wrap16(gL1.reshape(-1))),
            "mapl1": np.ascontiguousarray(_wrap16(mapLB1)),
        })
        unperm.append(pc2[r][1]["invperm"])  # node slot -> B2 position

    meta = dict(N=N, IN=IN, HID=HID, CLS=CLS, BLK=BLK, BLK_RAW=BLK_RAW,
                NPAD=NPAD, MT=MT, MTA=MTA, NROWSA=NROWSA, NROWSB=NROWSB,
                KA1=tuple(map(int, KA1)), KB1=tuple(map(int, KB1)),
                KA2=tuple(map(int, KA2)), KB2=tuple(map(int, KB2)),
                KL1=tuple(map(int, KL1)))
    return in_maps, meta, unperm


# ----------------------------------------------------------------------------
# Device graph
# ----------------------------------------------------------------------------

class _ScalarAdder:
    """Adapter: ACT-engine copy via activation identity."""

    def __init__(self, nc):
        self.nc = nc

    def tensor_copy(self, out, a):
        self.nc.scalar.activation(out, a,
                                  mybir.ActivationFunctionType.Identity)


def _tree_reduce_range(nc, g, lo, n, out_ap, eng):
    """Sum g[:, lo:lo+n, :] slabs; final level writes into out_ap."""
    if n == 1:
        eng.tensor_copy(out_ap, g[:, lo, :])
        return
    while n > 2:
        if n % 2 == 1:
            eng.tensor_add(g[:, lo, :], g[:, lo, :], g[:, lo + n - 1, :])
            n -= 1
            if n == 2:
                break
        h = n // 2
        eng.tensor_add(g[:, lo:lo + h, :], g[:, lo:lo + h, :],
                       g[:, lo + h:lo + 2 * h, :])
        n = h
    eng.tensor_add(out_ap, g[:, lo, :], g[:, lo + 1, :])


def _build(meta):
    IN, HID, CLS = meta["IN"], meta["HID"], meta["CLS"]
    BLK, MT, MTA = meta["BLK"], meta["MT"], meta["MTA"]
    NROWSA, NROWSB = meta["NROWSA"], meta["NROWSB"]
    KS = {1: (meta["KA1"], meta["KB1"]), 2: (meta["KA2"], meta["KB2"])}
    KL = meta["KL1"]
    WA = {1: max(1, sum(meta["KA1"])), 2: max(1, sum(meta["KA2"]))}
    WB = {1: max(1, sum(meta["KB1"])), 2: max(1, sum(meta["KB2"]))}
    WL = max(1, sum(KL))
    WAmax, WBmax = max(WA.values()), max(WB.values())
    KC = IN // P

    nc = bacc.Bacc("TRN2", target_bir_lowering=False, debug=False,
                   num_devices=N_CORES, num_swdge_queues=NQ)

    xt = nc.dram_tensor("xt", [MT * P, KC * P], dt.bfloat16,
                        kind="ExternalInput")
    w1 = nc.dram_tensor("w1", [IN, HID], dt.bfloat16, kind="ExternalInput")
    w2 = nc.dram_tensor("w2", [HID, HID], dt.bfloat16, kind="ExternalInput")
    wfc = nc.dram_tensor("wfc", [HID, CLS], dt.bfloat16, kind="ExternalInput")
    b1r = nc.dram_tensor("b1r", [P, HID], dt.float32, kind="ExternalInput")
    b2r = nc.dram_tensor("b2r", [P, HID], dt.float32, kind="ExternalInput")
    bfcr = nc.dram_tensor("bfcr", [P, CLS], dt.float32, kind="ExternalInput")
    dinv3 = nc.dram_tensor("dinv3", [P, 3, MT], dt.float32,
                           kind="ExternalInput")
    idx_dram = {}
    for layer in (1, 2):
        idx_dram[(layer, 0)] = nc.dram_tensor(
            f"idxa{layer}", [P, WA[layer] * 8], dt.int16, kind="ExternalInput")
        idx_dram[(layer, 1)] = nc.dram_tensor(
            f"idxb{layer}", [P, WB[layer] * 8], dt.int16, kind="ExternalInput")
    mapab = {1: nc.dram_tensor("mapab1", [P, BLK // 16], dt.int16,
                               kind="ExternalInput"),
             2: nc.dram_tensor("mapab2", [P, BLK // 16], dt.int16,
                               kind="ExternalInput")}
    idxl1 = nc.dram_tensor("idxl1", [P, WL * 8], dt.int16,
                           kind="ExternalInput")
    mapl1 = nc.dram_tensor("mapl1", [P, BLK // 16], dt.int16,
                           kind="ExternalInput")
    out = nc.dram_tensor("out", [BLK, CLS], dt.float32, kind="ExternalOutput")

    with tile.TileContext(nc) as tc:
        with (
            tc.tile_pool(name="const", bufs=1) as cpool,
            tc.tile_pool(name="idx", bufs=1) as ipool,
            tc.tile_pool(name="big", bufs=4) as bigpool,
            tc.tile_pool(name="xload", bufs=6) as xpool,
            tc.tile_pool(name="gbuf", bufs=GBUFS) as gpool,
            tc.tile_pool(name="lhsT", bufs=3) as tpool,
            tc.tile_pool(name="ps", bufs=4, space="PSUM") as pspool,
            tc.tile_pool(name="pst", bufs=1, space="PSUM") as pstpool,
            tc.tile_pool(name="psacc", bufs=2, space="PSUM") as psaccpool,
            tc.tile_pool(name="dram", bufs=1, space="DRAM") as dpool,
        ):
            # ---- constants ----
            w1sb = cpool.tile([P, KC, HID], dt.bfloat16, tag="w1")
            nc.sync.dma_start(out=w1sb[:], in_=w1[:].rearrange("(c k) h -> k c h", k=P))
            w2sb = cpool.tile([P, HID], dt.bfloat16, tag="w2")
            nc.sync.dma_start(out=w2sb[:], in_=w2[:])
            wfcsb = cpool.tile([P, CLS], dt.bfloat16, tag="wfc")
            nc.sync.dma_start(out=wfcsb[:], in_=wfc[:])
            b1sb = cpool.tile([P, HID], dt.float32, tag="b1")
            nc.sync.dma_start(out=b1sb[:], in_=b1r[:])
            b2sb = cpool.tile([P, HID], dt.float32, tag="b2")
            nc.sync.dma_start(out=b2sb[:], in_=b2r[:])
            bfcsb = cpool.tile([P, CLS], dt.float32, tag="bfc")
            nc.sync.dma_start(out=bfcsb[:], in_=bfcr[:])
            dvsb = cpool.tile([P, 3, MT], dt.float32, tag="dinv3")
            nc.sync.dma_start(out=dvsb[:], in_=dinv3[:])
            ident = cpool.tile([P, P], dt.float32, tag="ident")
            make_identity(nc, ident[:])
            identb = cpool.tile([P, P], dt.bfloat16, tag="identb")
            nc.any.tensor_copy(identb[:], ident[:])
            mapsb = {}
            for layer in (1, 2):
                mapsb[layer] = cpool.tile([P, BLK // 16], dt.int16,
                                          tag=f"map{layer}", name="mp")
                nc.sync.dma_start(out=mapsb[layer][:], in_=mapab[layer][:])

            idxasb = ipool.tile([P, WAmax * 8], dt.int16, tag="idxa")
            idxbsb = ipool.tile([P, WBmax * 8], dt.int16, tag="idxb")
            idxlsb = ipool.tile([P, WL * 8], dt.int16, tag="idxl")
            nc.sync.dma_start(out=idxasb[:, :WA[1] * 8], in_=idx_dram[(1, 0)][:])
            nc.sync.dma_start(out=idxbsb[:, :WB[1] * 8], in_=idx_dram[(1, 1)][:])
            nc.sync.dma_start(out=idxlsb[:], in_=idxl1[:])
            maplsb = cpool.tile([P, BLK // 16], dt.int16, tag="mapl",
                                name="mpl")
            nc.sync.dma_start(out=maplsb[:], in_=mapl1[:])

            self_q = [0]
            tile_n = [0]
            xmcache = {}
            QCH = max(1, (MT + 3) // 4)
            LT = min(2 * QCH, MT)

            def do_mm(layer, m, hsA, hsB, h_prev):
                dv_pre = dvsb[:, 0 if layer == 1 else 1, :]
                tgt = (hsA[:, m, :] if m < MTA or hsB is None
                       else hsB[:, m - MTA, :])
                ps = pspool.tile([P, HID], dt.float32, tag="mm", name="ps")
                if layer == 1:
                    xm = xpool.tile([P, KC, P], dt.bfloat16, tag="x",
                                    name="xm")
                    # ACT-engine HWDGE: keeps x-loads off the sync engine's
                    # serial DMA queue (agin/table/pd writes) at startup
                    nc.scalar.dma_start(
                        out=xm[:],
                        in_=xt[m * P:(m + 1) * P, :].rearrange(
                            "p (c j) -> p c j", c=KC),
                    )
                    for c in range(KC):
                        nc.tensor.matmul(
                            ps[:], xm[:, c, :], w1sb[:, c, :],
                            start=(c == 0), stop=(c == KC - 1),
                        )
                else:
                    pst = pstpool.tile([P, P], dt.bfloat16, tag="tr",
                                       name="pst")
                    nc.tensor.transpose(pst[:], h_prev[:, m, :], identb[:])
                    hT = tpool.tile([P, P], dt.bfloat16, tag="hT", name="hT")
                    nc.any.tensor_copy(hT[:], pst[:])
                    nc.tensor.matmul(ps[:], hT[:], w2sb[:], start=True,
                                     stop=True)
                nc.vector.tensor_scalar_mul(tgt, ps[:],
                                            dv_pre[:, m:m + 1])

            def emit_ag(layer, t0, t1, nrows, hseg, hseg_tile):
                agin = dpool.tile([(t1 - t0) * P, HID], dt.bfloat16,
                                  tag=f"agin{layer}{hseg}", name="agin")
                nc.sync.dma_start(
                    out=agin[:].rearrange("(t p) h -> p t h", p=P),
                    in_=hseg_tile[:, :t1 - t0, :],
                )
                tbl = dpool.tile([nrows, HID], dt.bfloat16,
                                 tag=f"table{layer}{hseg}", name="tbl",
                                 addr_space="Shared")
                nc.gpsimd.collective_compute(
                    "AllGather",
                    mybir.AluOpType.bypass,
                    replica_groups=[list(range(N_CORES))],
                    ins=[agin[:].opt()],
                    outs=[tbl[:].opt()],
                )
                return tbl

            def canon_map(msb, pdA, out_acc):
                for c0 in range(0, MT, CHUNK):
                    cc = min(CHUNK, MT - c0)
                    nc.gpsimd.dma_gather(
                        out_ap=out_acc[:, c0:c0 + cc, :], in_ap=pdA[:],
                        idxs_ap=msb[:, c0 * 8:(c0 + cc) * 8],
                        num_idxs=cc * P, num_idxs_reg=cc * P,
                        elem_size=HID, queue_num=self_q[0] % NQ,
                    )
                    self_q[0] += 1

            def canon_for(layer, pdA, out_acc):
                canon_map(mapsb[layer], pdA, out_acc)

            def merge_tiles(t0, t1, acc, partB, hnew, dv_post, bsb,
                            acc2=None):
                c0 = t0
                while c0 < t1:
                    c1 = min(t1, c0 + QCH)
                    sl = slice(c0, c1)
                    w = c1 - c0
                    src_h = partB
                    for a in (acc, acc2):
                        if a is not None:
                            nc.vector.tensor_add(hnew[:, sl, :], a[:, sl, :],
                                                 src_h[:, sl, :])
                            src_h = hnew
                    dv3b = dv_post[:, sl].to_broadcast([P, w, HID])
                    nc.vector.tensor_tensor(hnew[:, sl, :], src_h[:, sl, :],
                                            dv3b, op=mybir.AluOpType.mult)
                    b3 = bsb[:].rearrange(
                        "p (o h) -> p o h", o=1).to_broadcast([P, w, HID])
                    nc.vector.tensor_tensor(hnew[:, sl, :], hnew[:, sl, :],
                                            b3, op=mybir.AluOpType.add)
                    nc.scalar.activation(hnew[:, sl, :], hnew[:, sl, :],
                                         mybir.ActivationFunctionType.Relu)
                    c0 = c1

            # flat slab stream: every dma_gather carries a full 1024 idx;
            # hooks inject downstream work into the emission stream
            def gather_pass(Ks, isb, tview, part, hooks=()):
                offs = np.cumsum([0] + list(Ks))
                W = int(offs[-1])
                for t in range(MT):
                    if Ks[t] == 0:
                        nc.vector.memset(part[:, t, :], 0.0)
                pend = [[int(offs[min(tl, MT)]), fn] for tl, fn in hooks]
                state = {}
                g0 = 0
                while g0 < W:
                    while pend and g0 >= pend[0][0]:
                        pend.pop(0)[1]()
                    Kg = min(SCAP, W - g0)
                    gt = gpool.tile([P, Kg, HID], dt.bfloat16, tag="g")
                    g = gt[:]
                    for s0 in range(0, Kg, CHUNK):
                        kc = min(CHUNK, Kg - s0)
                        o0 = g0 + s0
                        nc.gpsimd.dma_gather(
                            out_ap=g[:, s0:s0 + kc, :],
                            in_ap=tview,
                            idxs_ap=isb[:, o0 * 8:(o0 + kc) * 8],
                            num_idxs=kc * P,
                            num_idxs_reg=kc * P,
                            elem_size=HID,
                            queue_num=self_q[0] % NQ,
                        )
                        self_q[0] += 1
                    t_lo = int(np.searchsorted(offs, g0, "right")) - 1
                    t_hi = int(np.searchsorted(offs, g0 + Kg, "left"))
                    for t in range(t_lo, t_hi):
                        lo = max(g0, int(offs[t])) - g0
                        hi = min(g0 + Kg, int(offs[t + 1])) - g0
                        if hi <= lo:
                            continue
                        st = state.get(t)
                        if st is None:
                            use_pe = tile_n[0] % PEROT == PEROT - 1
                            tile_n[0] += 1
                            psa = (psaccpool.tile([P, HID], dt.float32,
                                                  tag="acc", name="psa")
                                   if use_pe else None)
                            st = dict(first=True, pe=use_pe, psa=psa)
                            state[t] = st
                        tile_done = g0 + hi >= int(offs[t + 1])
                        if st["pe"]:
                            for s in range(lo, hi):
                                nc.tensor.matmul(
                                    st["psa"][:], identb[:], g[:, s, :],
                                    start=(st["first"] and s == lo),
                                    stop=(tile_done and s == hi - 1),
                                )
                            if tile_done:
                                _ScalarAdder(nc).tensor_copy(
                                    part[:, t, :], st["psa"][:])
                        elif st["first"]:
                            _tree_reduce_range(nc, g, lo, hi - lo,
                                               part[:, t, :], nc.vector)
                        else:
                            tmp = tpool.tile([P, P], dt.bfloat16, tag="gtmp")
                            _tree_reduce_range(nc, g, lo, hi - lo,
                                               tmp[:, :HID], nc.vector)
                            nc.vector.tensor_add(part[:, t, :],
                                                 part[:, t, :], tmp[:, :HID])
                        st["first"] = False
                    g0 += Kg
                for _, fn in pend:
                    fn()

            # ===== layer 1: dense + AGs (split hs so AG-A only depends
            # on half-A matmuls) =====
            tables1 = []
            if MTA > 0:
                hs1A = bigpool.tile([P, MTA, HID], dt.bfloat16, tag="big",
                                    name="hs1A")
            else:
                hs1A = None
            hs1B = bigpool.tile([P, MT - MTA, HID], dt.bfloat16, tag="big",
                                name="hs1B")
            if MTA > 0:
                for m in range(MTA):
                    do_mm(1, m, hs1A, hs1B, None)
                tables1.append(emit_ag(1, 0, MTA, NROWSA, 0, hs1A))
            else:
                tables1.append(None)
            for m in range(MTA, MT):
                do_mm(1, m, hs1A if hs1A is not None else hs1B, hs1B, None)
            tables1.append(emit_ag(1, MTA, MT, NROWSB, 1, hs1B))

            # ===== layer 1: gather passes =====
            KA, KB = KS[1]
            have_A1 = NROWSA > 0 and sum(KA) > 0
            if have_A1:
                partA1 = bigpool.tile([P, MT, HID], dt.bfloat16, tag="big",
                                      name="partA1")
                gather_pass(KA, idxasb, tables1[0][:], partA1)
                pdA1 = dpool.tile([BLK, HID], dt.bfloat16, tag="pdA1",
                                  name="pdA1")
                nc.sync.dma_start(
                    out=pdA1[:].rearrange("(t p) h -> p t h", p=P),
                    in_=partA1[:])
            partB1 = bigpool.tile([P, MT, HID], dt.bfloat16, tag="big",
                                  name="partB1")
            hnew1 = bigpool.tile([P, MT, HID], dt.bfloat16, tag="big",
                                 name="hnew1")
            accA2_1 = [None]
            hs2h = []
            tables2 = []

            def mid1():
                if have_A1:
                    accA2_1[0] = bigpool.tile([P, MT, HID], dt.bfloat16,
                                              tag="big", name="accA2x")
                    canon_for(1, pdA1, accA2_1[0])

            def late1():
                merge_tiles(0, LT, accA2_1[0], partB1, hnew1,
                            dvsb[:, 1, :], b1sb)
                if MTA > 0 and LT >= MTA:
                    hs2A = bigpool.tile([P, MTA, HID], dt.bfloat16,
                                        tag="big", name="hs2A")
                    hs2h.append(hs2A)
                    for m in range(MTA):
                        do_mm(2, m, hs2A, None, hnew1)
                    tables2.append(emit_ag(2, 0, MTA, NROWSA, 0, hs2A))

            gather_pass(KB, idxbsb, tables1[1][:], partB1,
                        hooks=((8, mid1), (LT, late1)))

            # prefetch layer-2 indices (in-place overwrite)
            nc.sync.dma_start(out=idxasb[:, :WA[2] * 8],
                              in_=idx_dram[(2, 0)][:])
            nc.sync.dma_start(out=idxbsb[:, :WB[2] * 8],
                              in_=idx_dram[(2, 1)][:])

            merge_tiles(LT, MT, accA2_1[0], partB1, hnew1, dvsb[:, 1, :],
                        b1sb)
            if not hs2h:
                if MTA > 0:
                    hs2A = bigpool.tile([P, MTA, HID], dt.bfloat16,
                                        tag="big", name="hs2Ab")
                    hs2h.append(hs2A)
                    for m in range(MTA):
                        do_mm(2, m, hs2A, None, hnew1)
                    tables2.append(emit_ag(2, 0, MTA, NROWSA, 0, hs2A))
                else:
                    tables2.append(None)
            hs2B = bigpool.tile([P, MT - MTA, HID], dt.bfloat16, tag="big",
                                name="hs2B")
            for m in range(MTA, MT):
                do_mm(2, m, hs2h[0] if hs2h else hs2B, hs2B, hnew1)
            tables2.append(emit_ag(2, MTA, MT, NROWSB, 1, hs2B))

            # ===== layer 2: gather passes + fc =====
            KA, KB = KS[2]
            have_A2 = NROWSA > 0 and sum(KA) > 0
            if have_A2:
                partA2 = bigpool.tile([P, MT, HID], dt.bfloat16, tag="big",
                                      name="partA2")
                gather_pass(KA, idxasb, tables2[0][:], partA2)
                pdA2 = dpool.tile([BLK, HID], dt.bfloat16, tag="pdA2",
                                  name="pdA2")
                nc.sync.dma_start(
                    out=pdA2[:].rearrange("(t p) h -> p t h", p=P),
                    in_=partA2[:])
            partB2 = bigpool.tile([P, MT, HID], dt.bfloat16, tag="big",
                                  name="partB2")
            hnew2 = bigpool.tile([P, MT, HID], dt.bfloat16, tag="big",
                                 name="hnew2")
            outsb = bigpool.tile([P, MT, CLS], dt.float32, tag="big",
                                 name="outsb")
            accA2_2 = [None]

            def mid2():
                if have_A2:
                    accA2_2[0] = bigpool.tile([P, MT, HID], dt.bfloat16,
                                              tag="big", name="accA2y")
                    canon_for(2, pdA2, accA2_2[0])

            def fc_tiles(t0, t1):
                for m in range(t0, t1):
                    pst = pstpool.tile([P, P], dt.bfloat16, tag="tr",
                                       name="pst")
                    nc.tensor.transpose(pst[:], hnew2[:, m, :], identb[:])
                    hT = tpool.tile([P, P], dt.bfloat16, tag="hT", name="hT")
                    nc.any.tensor_copy(hT[:], pst[:])
                    ps2 = pspool.tile([P, CLS], dt.float32, tag="mm",
                                      name="ps2")
                    nc.tensor.matmul(ps2[:], hT[:], wfcsb[:], start=True,
                                     stop=True)
                    nc.vector.tensor_add(outsb[:, m, :], ps2[:], bfcsb[:])
                if t1 > t0:
                    nc.sync.dma_start(
                        out=out[:].rearrange("(t p) c -> p t c", p=P)[
                            :, t0:t1, :],
                        in_=outsb[:, t0:t1, :],
                    )

            def late2():
                merge_tiles(0, LT, accA2_2[0], partB2, hnew2,
                            dvsb[:, 2, :], b2sb)
                fc_tiles(0, LT)

            LT2 = min(max(LT, (LT + MT) // 2 + 3), MT)

            def late2b():
                merge_tiles(LT, LT2, accA2_2[0], partB2, hnew2,
                            dvsb[:, 2, :], b2sb)
                fc_tiles(LT, LT2)

            gather_pass(KB, idxbsb, tables2[1][:], partB2,
                        hooks=((8, mid2), (LT, late2), (LT2, late2b)))
            merge_tiles(LT2, MT, accA2_2[0], partB2, hnew2, dvsb[:, 2, :],
                        b2sb)
            fc_tiles(LT2, MT)

    nc.compile()
    return nc


# ----------------------------------------------------------------------------
# Entry point
# ----------------------------------------------------------------------------

_CACHE = {}


def _get_graph(meta):
    key = (meta["IN"], meta["HID"], meta["CLS"], meta["BLK"], meta["NPAD"],
           meta["KA1"], meta["KB1"], meta["KA2"], meta["KB2"], meta["KL1"])
    if key not in _CACHE:
        _CACHE[key] = _build(meta)
    return _CACHE[key]


def kernel(x, edge_index, W1, b1, W2, b2, Wfc, bfc, _want_profile=False):
    x = np.asarray(x, dtype=np.float32)
    in_maps, meta, unperm = _preprocess(
        np.asarray(x), np.asarray(edge_index), np.asarray(W1), np.asarray(b1),
        np.asarray(W2), np.asarray(b2), np.asarray(Wfc), np.asarray(bfc))
    nc = _get_graph(meta)
    res = run_bass_kernel_spmd(nc, in_maps, core_ids=list(range(N_CORES)),
                               trace=_want_profile)
    N, CLS = meta["N"], meta["CLS"]
    BLK_RAW = meta["BLK_RAW"]
    full = np.empty((N, CLS), dtype=np.float32)
    for r in range(N_CORES):
        lo = r * BLK_RAW
        hi = min(N, (r + 1) * BLK_RAW)
        if hi > lo:
            rows = unperm[r][1:1 + hi - lo]  # canonical slot j -> B2 position
            full[lo:hi] = res.results[r]["out"][rows]
    if _want_profile:
        return full, res
    return full
